# revision 7
# baseline (speedup 1.0000x reference)
"""Trainium2 Bass kernel for nn_ContrastiveLoss (circular-shift negatives).

Reference computation (B=4096, D=1024, S=5):
    d_p[k]      = ||v[k] - a[k] + eps||
    d_n1[k,m]   = ||v[k] - a[idx(k,m)] + eps||,  idx(k,m) = (k+m+1)%B  (m==k -> (k+1)%B)
    d_n2[k,m]   = ||a[k] - v[idx(k,m)] + eps||
    loss        = mean(relu(1 + 2*d_p - min_m d_n1 - min_m d_n2))

Strategy (8 cores, data-parallel over batch, 512 anchors/core + 5-row halo):
  - All distances via the norm expansion ||x-y||^2 = ||x||^2 + ||y||^2 - 2<x,y>.
    (The +eps inside the norm perturbs d^2 by ~1e-4 relative 2e-8 -> dropped;
     effect on the result is ~1e-6 relative, far below tolerance.)
  - <v[k], a[j]> for the band j in [k, k+5] plus row norms come from PE
    matmuls over transposed tiles: band1 = [V.A^T | V.V^T], band2 = [A.A^T | A.V^T]
    computed per 128-anchor block with a 2-group rhs access pattern (N=266)
    so float32r runs at 1 cycle/row.
  - Diagonal extraction: bands are bounced SBUF->DRAM, then strided DMA
    gathers (stride 267 elements) pull the 11 diagonals into lane-aligned
    [128, m] tiles. Small vector/scalar epilogue computes the hinge.
  - Anchors k<5 (where m==k rewrites the negative index) are recomputed
    exactly on the host in numpy and spliced in.
"""

import numpy as np

B, D, S = 4096, 1024, 5
NCORES = 8
SH = B // NCORES          # 512 anchors per core
ROWS = SH + S             # 517 rows needed per shard (incl. halo)
MARGIN = 1.0
EPS = 1e-6

_CACHE = {}


def _build():
    import concourse.bass as bass
    import concourse.bacc as bacc
    import concourse.tile as tile
    import concourse.mybir as mybir
    from concourse.masks import make_identity

    f32 = mybir.dt.float32
    f32r = mybir.dt.float32r

    nc = bacc.Bacc()
    v_ext = nc.declare_dram_parameter("v", [ROWS, D], f32, isOutput=False)
    a_ext = nc.declare_dram_parameter("a", [ROWS, D], f32, isOutput=False)
    loss_ext = nc.declare_dram_parameter("loss", [SH], f32, isOutput=True)

    NB = SH // 128            # 4 anchor blocks per core
    NC = D // 128             # 8 contraction chunks
    W = 520                   # column pitch of one tensor group in T_all
    BW = 133                  # band width per group (128 anchors + 5 halo)
    N2 = 2 * BW               # 266 = matmul moving free size (>=256 -> f32r fast)

    from contextlib import ExitStack

    with tile.TileContext(nc) as tc, ExitStack() as ctx:
        sing = ctx.enter_context(tc.tile_pool(name="sing", bufs=1))
        natp = ctx.enter_context(tc.tile_pool(name="natp", bufs=1))
        tallp = ctx.enter_context(tc.tile_pool(name="tall", bufs=1))
        tpsum = ctx.enter_context(tc.tile_pool(name="tpsum", bufs=2, space="PSUM"))
        hpsum = ctx.enter_context(tc.tile_pool(name="hpsum", bufs=2, space="PSUM"))
        bpsum = ctx.enter_context(tc.tile_pool(name="bpsum", bufs=3, space="PSUM"))
        bsb = ctx.enter_context(tc.tile_pool(name="bsb", bufs=3))
        dramp = ctx.enter_context(tc.tile_pool(name="dramp", bufs=2, space="DRAM"))
        hdram = ctx.enter_context(tc.tile_pool(name="hdram", bufs=1, space="DRAM"))
        ndram = ctx.enter_context(tc.tile_pool(name="ndram", bufs=1, space="DRAM"))
        ep = ctx.enter_context(tc.tile_pool(name="ep", bufs=1))

        identity = sing.tile([128, 128], f32, tag="ident")
        make_identity(nc, identity)

        # T_all[p, c, 2j+s]: s=0 -> A^T col j, s=1 -> V^T col j (interleaved
        # so each block's 266-wide rhs window is one contiguous slice, as
        # required by the fp32r matmul ISA restrictions).
        t_all = tallp.tile([128, NC, 2 * W], f32r, tag="t_all")
        t_view = t_all[:, :, :].rearrange("p c (j s) -> p c j s", s=2)

        # ---- load natural tiles + transpose into T_all ----
        nats = {}
        for ti, ext in ((0, a_ext), (1, v_ext)):
            for rg in range(5):
                rows = 128 if rg < 4 else ROWS - 512
                nat = natp.tile([128, D], f32, tag=f"nat{ti}_{rg}")
                nc.sync.dma_start(out=nat[:rows, :], in_=ext[rg * 128 : rg * 128 + rows, :])
                nats[(ti, rg)] = nat

        for ti in range(2):
            s_off = ti  # 0 = A^T, 1 = V^T
            for c in range(NC):
                ps = tpsum.tile([128, 512], f32, tag="tps")
                for rg in range(4):
                    nc.tensor.transpose(
                        ps[:, rg * 128 : (rg + 1) * 128],
                        nats[(ti, rg)][:, c * 128 : (c + 1) * 128],
                        identity,
                    )
                dst = t_view[:, c, 0:512, s_off]
                if c % 2 == 0:
                    nc.vector.tensor_copy(dst, ps[:, :])
                else:
                    nc.scalar.copy(dst, ps[:, :])
            # halo rows 512..516 -> cols 512..516 of each chunk
            hps = hpsum.tile([128, 5 * NC], f32, tag="hps")
            for c in range(NC):
                nc.tensor.transpose(
                    hps[:, c * 5 : c * 5 + 5],
                    nats[(ti, 4)][:5, c * 128 : (c + 1) * 128],
                    identity[:5, :5],
                )
            for c in range(NC):
                dsth = t_view[:, c, 512:517, s_off]
                if c % 2 == 0:
                    nc.vector.tensor_copy(dsth, hps[:, c * 5 : c * 5 + 5])
                else:
                    nc.scalar.copy(dsth, hps[:, c * 5 : c * 5 + 5])

        def band_matmul(lhs_j0, lhs_s, lhs_n, k0, tag):
            """PSUM [lhs_n, 266] = sum_c lhsT_c.T @ interleaved[A^T|V^T] window.

            lhsT = rows lhs_j0..lhs_j0+lhs_n of tensor lhs_s (0=A, 1=V);
            rhs  = contiguous cols 2*k0 .. 2*k0+265 (row-pairs k0..k0+132).
            Output col 2c+s = <lhs_row, (A if s==0 else V)[k0+c]>.
            """
            bp = bpsum.tile([128, N2], f32, tag="band")
            for c in range(NC):
                lhsT = t_view[:, c, lhs_j0 : lhs_j0 + lhs_n, lhs_s]
                rhs = t_all[:, c, 2 * k0 : 2 * k0 + N2]
                nc.tensor.matmul(bp[:lhs_n, :], lhsT, rhs, start=(c == 0), stop=(c == NC - 1))
            return bp

        def to_dram(bp, rows, tag, pool):
            sb = bsb.tile([128, N2], f32, tag="bsb")
            eng = nc.vector if (hash(tag) % 2 == 0) else nc.scalar
            if eng is nc.vector:
                eng.tensor_copy(sb[:rows, :], bp[:rows, :])
            else:
                eng.copy(sb[:rows, :], bp[:rows, :])
            dt_ = pool.tile([rows, N2], f32, tag=tag)
            nc.sync.dma_start(out=dt_[:, :], in_=sb[:rows, :])
            return dt_

        # ---- norms DRAM staging ----
        nv_d = ndram.tile([ROWS + 3], f32, tag="nv_d")
        na_d = ndram.tile([ROWS + 3], f32, tag="na_d")

        def diag_ap(dram_tile, col0, n, width, elem_step=2, row_step=268):
            return bass.AP(
                tensor=dram_tile.tensor,
                offset=dram_tile.offset + col0,
                ap=[[row_step, n], [elem_step, width]],
            )

        def vec_ap(dram_tile, i0, dims):
            return bass.AP(tensor=dram_tile.tensor, offset=dram_tile.offset + i0, ap=dims)

        # epilogue batched tiles
        g1 = ep.tile([128, NB, 6], f32, tag="g1")      # dp dot + 5 dn1 dots
        g2 = ep.tile([128, NB, 5], f32, tag="g2")      # 5 dn2 dots
        nv_t = ep.tile([128, NB, 1], f32, tag="nv")    # ||v[k]||^2
        na_t = ep.tile([128, NB, 1], f32, tag="na")    # ||a[k]||^2
        nash = ep.tile([128, NB, 5], f32, tag="nash")  # ||a[k+1+m]||^2
        nvsh = ep.tile([128, NB, 5], f32, tag="nvsh")  # ||v[k+1+m]||^2

        band_drams = []
        for b in range(NB):
            k0 = 128 * b
            bp1 = band_matmul(k0, 1, 128, k0, f"b1_{b}")   # lhsT=V -> VA | VV
            bp2 = band_matmul(k0, 0, 128, k0, f"b2_{b}")   # lhsT=A -> AA | AV
            bd1 = to_dram(bp1, 128, "bd1", dramp)
            bd2 = to_dram(bp2, 128, "bd2", dramp)
            band_drams.append((bd1, bd2))

            # gathers from band1: dp+dn1 dots (cols k..k+5), VV diag (col 133+k)
            nc.sync.dma_start(out=g1[:, b, :], in_=diag_ap(bd1, 0, 128, 6))  # dp + dn1 dots (cols 2k,2k+2..2k+10)
            nc.sync.dma_start(out=nv_t[:, b, :], in_=diag_ap(bd1, 1, 128, 1))  # VV diag (col 2k+1)
            # gathers from band2: dn2 dots (cols 134+k..138+k), AA diag (col k)
            nc.sync.dma_start(out=g2[:, b, :], in_=diag_ap(bd2, 3, 128, 5))  # dn2 dots (cols 2k+3..2k+11)
            nc.sync.dma_start(out=na_t[:, b, :], in_=diag_ap(bd2, 0, 128, 1))  # AA diag (col 2k)
            # norm writebacks to flat staging vectors
            nc.sync.dma_start(out=vec_ap(nv_d, k0, [[1, 128]]), in_=nv_t[:, b, :])
            nc.sync.dma_start(out=vec_ap(na_d, k0, [[1, 128]]), in_=na_t[:, b, :])

        # halo norms: rows 512..516 via tiny band matmuls at k0=384
        bp4 = band_matmul(512, 1, 5, 384, "b4")    # V halo: VV diag at col 257+2i
        bp5 = band_matmul(512, 0, 5, 384, "b5")    # A halo: AA diag at col 256+2i
        bd4 = to_dram(bp4, 5, "bd4", hdram)
        bd5 = to_dram(bp5, 5, "bd5", hdram)
        nvh = ep.tile([5, 1], f32, tag="nvh")
        nah = ep.tile([5, 1], f32, tag="nah")
        nc.sync.dma_start(out=nvh[:, :], in_=diag_ap(bd4, 257, 5, 1))
        nc.sync.dma_start(out=nah[:, :], in_=diag_ap(bd5, 256, 5, 1))
        nc.sync.dma_start(out=vec_ap(nv_d, 512, [[1, 5]]), in_=nvh[:, :])
        nc.sync.dma_start(out=vec_ap(na_d, 512, [[1, 5]]), in_=nah[:, :])

        # shifted norm gathers (read across block boundaries incl. halo)
        for b in range(NB):
            k0 = 128 * b
            nc.sync.dma_start(out=nash[:, b, :], in_=vec_ap(na_d, k0 + 1, [[1, 128], [1, 5]]))
            nc.sync.dma_start(out=nvsh[:, b, :], in_=vec_ap(nv_d, k0 + 1, [[1, 128], [1, 5]]))

        # ---- epilogue ----
        dsq = ep.tile([128, NB, 11], f32, tag="dsq")
        dall = ep.tile([128, NB, 11], f32, tag="dall")
        dn1m = ep.tile([128, NB], f32, tag="dn1m")
        dn2m = ep.tile([128, NB], f32, tag="dn2m")
        tsum = ep.tile([128, NB], f32, tag="tsum")
        hpre = ep.tile([128, NB], f32, tag="hpre")
        lossn = ep.tile([128, NB], f32, tag="lossn")

        A = mybir.AluOpType
        nvb = nv_t[:, :, :].broadcast_to([128, NB, 5])
        nab = na_t[:, :, :].broadcast_to([128, NB, 5])

        # dn1^2 = -2*dot + nv + na_shift
        nc.vector.tensor_scalar(out=dsq[:, :, 0:5], in0=g1[:, :, 1:6], scalar1=-2.0,
                                scalar2=None, op0=A.mult)
        nc.vector.tensor_add(dsq[:, :, 0:5], dsq[:, :, 0:5], nvb)
        nc.vector.tensor_add(dsq[:, :, 0:5], dsq[:, :, 0:5], nash[:, :, :])
        # dn2^2 = -2*dot + na + nv_shift
        nc.vector.tensor_scalar(out=dsq[:, :, 5:10], in0=g2[:, :, :], scalar1=-2.0,
                                scalar2=None, op0=A.mult)
        nc.vector.tensor_add(dsq[:, :, 5:10], dsq[:, :, 5:10], nab)
        nc.vector.tensor_add(dsq[:, :, 5:10], dsq[:, :, 5:10], nvsh[:, :, :])
        # dp^2 = -2*dot + nv + na
        nc.vector.tensor_scalar(out=dsq[:, :, 10:11], in0=g1[:, :, 0:1], scalar1=-2.0,
                                scalar2=None, op0=A.mult)
        nc.vector.tensor_add(dsq[:, :, 10:11], dsq[:, :, 10:11], nv_t[:, :, :])
        nc.vector.tensor_add(dsq[:, :, 10:11], dsq[:, :, 10:11], na_t[:, :, :])
        # clamp tiny negatives from rounding before sqrt
        nc.vector.tensor_scalar_max(out=dsq[:, :, :], in0=dsq[:, :, :], scalar1=0.0)
        nc.scalar.activation(out=dall[:, :, :], in_=dsq[:, :, :],
                             func=mybir.ActivationFunctionType.Sqrt)
        nc.vector.tensor_reduce(out=dn1m[:, :], in_=dall[:, :, 0:5],
                                axis=mybir.AxisListType.X, op=A.min)
        nc.vector.tensor_reduce(out=dn2m[:, :], in_=dall[:, :, 5:10],
                                axis=mybir.AxisListType.X, op=A.min)
        nc.vector.tensor_add(tsum[:, :], dn1m[:, :], dn2m[:, :])
        nc.vector.tensor_scalar(out=hpre[:, :], in0=dall[:, :, 10], scalar1=2.0,
                                scalar2=MARGIN, op0=A.mult, op1=A.add)
        nc.vector.tensor_sub(lossn[:, :], hpre[:, :], tsum[:, :])
        nc.vector.tensor_scalar_max(out=lossn[:, :], in0=lossn[:, :], scalar1=0.0)

        loss_dst = bass.AP(tensor=loss_ext, offset=0, ap=[[1, 128], [128, NB]])
        nc.sync.dma_start(out=loss_dst, in_=lossn[:, :])

    nc.finalize()
    return nc


def _exact_losses_head(vfeat, afeat, ks):
    """Exact reference loss for anchors in ks (handles the m==k index rewrite)."""
    v = vfeat.astype(np.float64)
    a = afeat.astype(np.float64)
    out = []
    for k in ks:
        idx = [(m + k + 1) % B if m != k else (k + 1) % B for m in range(S)]
        d_p = np.sqrt(np.sum((v[k] - a[k] + EPS) ** 2))
        d1 = min(np.sqrt(np.sum((v[k] - a[j] + EPS) ** 2)) for j in idx)
        d2 = min(np.sqrt(np.sum((a[k] - v[j] + EPS) ** 2)) for j in idx)
        out.append(max(MARGIN + 2.0 * d_p - d1 - d2, 0.0))
    return out


def run_kernel(vfeat, afeat, trace=False):
    from concourse.bass_utils import run_bass_kernel_spmd

    vfeat = np.ascontiguousarray(np.asarray(vfeat, dtype=np.float32))
    afeat = np.ascontiguousarray(np.asarray(afeat, dtype=np.float32))

    if "nc" not in _CACHE:
        _CACHE["nc"] = _build()
    nc = _CACHE["nc"]

    in_maps = []
    for c in range(NCORES):
        lo = c * SH
        idx = np.arange(lo, lo + ROWS) % B
        in_maps.append({"v": vfeat[idx], "a": afeat[idx]})

    res = run_bass_kernel_spmd(nc, in_maps, core_ids=list(range(NCORES)), trace=trace)
    losses = np.concatenate([res.results[c]["loss"] for c in range(NCORES)])

    total = float(np.sum(losses[S:], dtype=np.float64))
    total += sum(_exact_losses_head(vfeat, afeat, range(S)))
    mean = np.float32(total / B)
    return np.asarray(mean, dtype=np.float32), res


def kernel(vfeat, afeat):
    out, _ = run_kernel(vfeat, afeat, trace=False)
    return out


# revision 10
# speedup vs baseline: 1.2591x; 1.2591x over previous
"""Trainium2 Bass kernel for nn_ContrastiveLoss (circular-shift negatives).

Reference computation (B=4096, D=1024, S=5):
    d_p[k]      = ||v[k] - a[k] + eps||
    d_n1[k,m]   = ||v[k] - a[idx(k,m)] + eps||,  idx(k,m) = (k+m+1)%B  (m==k -> (k+1)%B)
    d_n2[k,m]   = ||a[k] - v[idx(k,m)] + eps||
    loss        = mean(relu(1 + 2*d_p - min_m d_n1 - min_m d_n2))

Strategy (8 cores, data-parallel over batch, 512 anchors/core + 5-row halo):
  - All distances via the norm expansion ||x-y||^2 = ||x||^2 + ||y||^2 - 2<x,y>.
    (The +eps inside the norm perturbs d^2 by ~1e-4 relative 2e-8 -> dropped;
     effect on the result is ~1e-6 relative, far below tolerance.)
  - <v[k], a[j]> for the band j in [k, k+5] plus row norms come from PE
    matmuls over transposed tiles: band1 = [V.A^T | V.V^T], band2 = [A.A^T | A.V^T]
    computed per 128-anchor block with a 2-group rhs access pattern (N=266)
    so float32r runs at 1 cycle/row.
  - Diagonal extraction: bands are bounced SBUF->DRAM, then strided DMA
    gathers (stride 267 elements) pull the 11 diagonals into lane-aligned
    [128, m] tiles. Small vector/scalar epilogue computes the hinge.
  - Anchors k<5 (where m==k rewrites the negative index) are recomputed
    exactly on the host in numpy and spliced in.
"""

import numpy as np

B, D, S = 4096, 1024, 5
NCORES = 8
SH = B // NCORES          # 512 anchors per core
ROWS = SH + S             # 517 rows needed per shard (incl. halo)
MARGIN = 1.0
EPS = 1e-6

_CACHE = {}


def _build():
    import concourse.bass as bass
    import concourse.bacc as bacc
    import concourse.tile as tile
    import concourse.mybir as mybir
    from concourse.masks import make_identity

    f32 = mybir.dt.float32
    f32r = mybir.dt.float32r

    nc = bacc.Bacc()
    v_ext = nc.declare_dram_parameter("v", [ROWS, D], f32, isOutput=False)
    a_ext = nc.declare_dram_parameter("a", [ROWS, D], f32, isOutput=False)
    loss_ext = nc.declare_dram_parameter("loss", [SH], f32, isOutput=True)

    NB = SH // 128            # 4 anchor blocks per core
    NC = D // 128             # 8 contraction chunks
    W = 520                   # column pitch of one tensor group in T_all
    BW = 133                  # band width per group (128 anchors + 5 halo)
    N2 = 2 * BW               # 266 = matmul moving free size (>=256 -> f32r fast)

    from contextlib import ExitStack

    with tile.TileContext(nc) as tc, ExitStack() as ctx:
        sing = ctx.enter_context(tc.tile_pool(name="sing", bufs=1))
        natp = ctx.enter_context(tc.tile_pool(name="natp", bufs=1))
        tallp = ctx.enter_context(tc.tile_pool(name="tall", bufs=1))
        tpsum = ctx.enter_context(tc.tile_pool(name="tpsum", bufs=2, space="PSUM"))
        hpsum = ctx.enter_context(tc.tile_pool(name="hpsum", bufs=2, space="PSUM"))
        bpsum = ctx.enter_context(tc.tile_pool(name="bpsum", bufs=3, space="PSUM"))
        bsb = ctx.enter_context(tc.tile_pool(name="bsb", bufs=3))
        dramp = ctx.enter_context(tc.tile_pool(name="dramp", bufs=2, space="DRAM"))
        hdram = ctx.enter_context(tc.tile_pool(name="hdram", bufs=1, space="DRAM"))
        ndram = ctx.enter_context(tc.tile_pool(name="ndram", bufs=1, space="DRAM"))
        ep = ctx.enter_context(tc.tile_pool(name="ep", bufs=1))

        identity = sing.tile([128, 128], f32, tag="ident")
        make_identity(nc, identity)

        # T_all[p, c, 2j+s]: s=0 -> A^T col j, s=1 -> V^T col j (interleaved
        # so each block's 266-wide rhs window is one contiguous slice, as
        # required by the fp32r matmul ISA restrictions).
        t_all = tallp.tile([128, NC, 2 * W], f32r, tag="t_all")
        t_view = t_all[:, :, :].rearrange("p c (j s) -> p c j s", s=2)

        # ---- load natural tiles (merged DMAs) ----
        nats = {}
        halos = {}
        for ti, ext in ((0, a_ext), (1, v_ext)):
            e1, e2 = (nc.sync, nc.scalar) if ti == 0 else (nc.scalar, nc.sync)
            nat = natp.tile([128, 4, D], f32, tag=f"nat{ti}")
            src_main = ext[0:512, :].rearrange("(g p) d -> p g d", p=128)
            e1.dma_start(out=nat[:, 0:2, :], in_=src_main[:, 0:2, :])
            e2.dma_start(out=nat[:, 2:4, :], in_=src_main[:, 2:4, :])
            halo = natp.tile([128, D], f32, tag=f"halo{ti}")
            e1.dma_start(out=halo[:5, :], in_=ext[512:ROWS, :])
            nats[ti] = nat
            halos[ti] = halo

        # halo row norms (rows 512..516) directly from natural layout
        nvh = ep.tile([128, 1], f32, tag="nvh")
        nah = ep.tile([128, 1], f32, tag="nah")
        scr_h = ep.tile([128, D], f32, tag="scr_h")
        scr_h2 = ep.tile([128, D], f32, tag="scr_h2")
        A = mybir.AluOpType
        nc.scalar.activation(out=scr_h[:5, :], in_=halos[0][:5, :],
                             func=mybir.ActivationFunctionType.Square,
                             accum_out=nah[:5, :])
        nc.scalar.activation(out=scr_h2[:5, :], in_=halos[1][:5, :],
                             func=mybir.ActivationFunctionType.Square,
                             accum_out=nvh[:5, :])

        # ---- transpose into T_all ----
        for ti in range(2):
            s_off = ti  # 0 = A^T, 1 = V^T
            for c in range(NC):
                ps = tpsum.tile([128, 512], f32, tag="tps")
                for rg in range(4):
                    nc.tensor.transpose(
                        ps[:, rg * 128 : (rg + 1) * 128],
                        nats[ti][:, rg, c * 128 : (c + 1) * 128],
                        identity,
                    )
                dst = t_view[:, c, 0:512, s_off]
                if c % 2 == 0:
                    nc.vector.tensor_copy(dst, ps[:, :])
                else:
                    nc.scalar.copy(dst, ps[:, :])
            # halo rows 512..516 -> cols 512..516 of every chunk (one copy)
            hps = hpsum.tile([128, 5 * NC], f32, tag="hps")
            for c in range(NC):
                nc.tensor.transpose(
                    hps[:, c * 5 : c * 5 + 5],
                    halos[ti][:5, c * 128 : (c + 1) * 128],
                    identity[:5, :5],
                )
            hview = hps[:, :].rearrange("p (c j) -> p c j", j=5)
            dsth = t_view[:, :, 512:517, s_off]
            if ti == 0:
                nc.vector.tensor_copy(dsth, hview)
            else:
                nc.scalar.copy(dsth, hview)

        def band_matmul(lhs_j0, lhs_s, lhs_n, k0, tag):
            """PSUM [lhs_n, 266] = sum_c lhsT_c.T @ interleaved[A^T|V^T] window.

            lhsT = rows lhs_j0..lhs_j0+lhs_n of tensor lhs_s (0=A, 1=V);
            rhs  = contiguous cols 2*k0 .. 2*k0+265 (row-pairs k0..k0+132).
            Output col 2c+s = <lhs_row, (A if s==0 else V)[k0+c]>.
            """
            bp = bpsum.tile([128, N2], f32, tag="band")
            for c in range(NC):
                lhsT = t_view[:, c, lhs_j0 : lhs_j0 + lhs_n, lhs_s]
                rhs = t_all[:, c, 2 * k0 : 2 * k0 + N2]
                nc.tensor.matmul(bp[:lhs_n, :], lhsT, rhs, start=(c == 0), stop=(c == NC - 1))
            return bp

        # band accumulators in SBUF, dumped to DRAM with one DMA each
        b1acc = ep.tile([128, NB, N2], f32, tag="b1acc")
        b2acc = ep.tile([128, NB, N2], f32, tag="b2acc")
        for b in range(NB):
            k0 = 128 * b
            bp1 = band_matmul(k0, 1, 128, k0, f"b1_{b}")   # lhsT=V -> VA | VV
            bp2 = band_matmul(k0, 0, 128, k0, f"b2_{b}")   # lhsT=A -> AA | AV
            nc.vector.tensor_copy(b1acc[:, b, :], bp1[:, :])
            nc.scalar.copy(b2acc[:, b, :], bp2[:, :])

        bd1 = dramp.tile([128, NB, N2], f32, tag="bd1")
        bd2 = dramp.tile([128, NB, N2], f32, tag="bd2")
        nc.sync.dma_start(out=bd1[:, :, :], in_=b1acc[:, :, :])
        nc.scalar.dma_start(out=bd2[:, :, :], in_=b2acc[:, :, :])

        # ---- diagonal gathers (strided DMA from the DRAM bounce) ----
        # bd layout flat(p, b, c) = 1064*p + 266*b + c; diagonal element
        # (p, b, j) sits at c = 2p + j  ->  flat = 1066*p + 266*b + j.
        def band_gather(bdt, width):
            return bass.AP(
                tensor=bdt.tensor,
                offset=bdt.offset,
                ap=[[1066, 128], [266, NB], [1, width]],
            )

        def vec_ap(dram_tile, i0, dims):
            return bass.AP(tensor=dram_tile.tensor, offset=dram_tile.offset + i0, ap=dims)

        # g1: j=0 dp dot, j=1 VV diag (nv), j=2,4,6,8,10 dn1 dots
        g1 = ep.tile([128, NB, 11], f32, tag="g1")
        # g2: j=0 AA diag (na), j=3,5,7,9,11 dn2 dots
        g2 = ep.tile([128, NB, 12], f32, tag="g2")
        nc.sync.dma_start(out=g1[:, :, :], in_=band_gather(bd1, 11))
        nc.scalar.dma_start(out=g2[:, :, :], in_=band_gather(bd2, 12))

        # ---- norm staging vector + shifted gathers ----
        nv_d = ndram.tile([ROWS + 3], f32, tag="nv_d")
        na_d = ndram.tile([ROWS + 3], f32, tag="na_d")
        nc.sync.dma_start(out=vec_ap(nv_d, 0, [[1, 128], [128, NB]]), in_=g1[:, :, 1])
        nc.scalar.dma_start(out=vec_ap(na_d, 0, [[1, 128], [128, NB]]), in_=g2[:, :, 0])
        nc.sync.dma_start(out=vec_ap(nv_d, 512, [[1, 5]]), in_=nvh[:5, :])
        nc.scalar.dma_start(out=vec_ap(na_d, 512, [[1, 5]]), in_=nah[:5, :])

        nash = ep.tile([128, NB, 5], f32, tag="nash")  # ||a[k+1+m]||^2
        nvsh = ep.tile([128, NB, 5], f32, tag="nvsh")  # ||v[k+1+m]||^2
        nc.sync.dma_start(out=nash[:, :, :], in_=vec_ap(na_d, 1, [[1, 128], [128, NB], [1, 5]]))
        nc.scalar.dma_start(out=nvsh[:, :, :], in_=vec_ap(nv_d, 1, [[1, 128], [128, NB], [1, 5]]))

        # ---- epilogue ----
        dsq = ep.tile([128, NB, 11], f32, tag="dsq")
        dall = ep.tile([128, NB, 11], f32, tag="dall")
        dn1m = ep.tile([128, NB], f32, tag="dn1m")
        dn2m = ep.tile([128, NB], f32, tag="dn2m")
        tsum = ep.tile([128, NB], f32, tag="tsum")
        hpre = ep.tile([128, NB], f32, tag="hpre")
        lossn = ep.tile([128, NB], f32, tag="lossn")

        nv_t = g1[:, :, 1:2]
        na_t = g2[:, :, 0:1]
        dn1_dots = g1[:, :, 1:11].rearrange("p b (j s) -> p b j s", s=2)[:, :, :, 1]
        dn2_dots = g2[:, :, 2:12].rearrange("p b (j s) -> p b j s", s=2)[:, :, :, 1]
        nvb = nv_t.broadcast_to([128, NB, 5])
        nab = na_t.broadcast_to([128, NB, 5])

        # dn1^2 = -2*dot + nv + na_shift
        nc.vector.tensor_scalar(out=dsq[:, :, 0:5], in0=dn1_dots, scalar1=-2.0,
                                scalar2=None, op0=A.mult)
        nc.vector.tensor_add(dsq[:, :, 0:5], dsq[:, :, 0:5], nvb)
        nc.vector.tensor_add(dsq[:, :, 0:5], dsq[:, :, 0:5], nash[:, :, :])
        # dn2^2 = -2*dot + na + nv_shift
        nc.vector.tensor_scalar(out=dsq[:, :, 5:10], in0=dn2_dots, scalar1=-2.0,
                                scalar2=None, op0=A.mult)
        nc.vector.tensor_add(dsq[:, :, 5:10], dsq[:, :, 5:10], nab)
        nc.vector.tensor_add(dsq[:, :, 5:10], dsq[:, :, 5:10], nvsh[:, :, :])
        # dp^2 = -2*dot + nv + na
        nc.vector.tensor_scalar(out=dsq[:, :, 10:11], in0=g1[:, :, 0:1], scalar1=-2.0,
                                scalar2=None, op0=A.mult)
        nc.vector.tensor_add(dsq[:, :, 10:11], dsq[:, :, 10:11], nv_t)
        nc.vector.tensor_add(dsq[:, :, 10:11], dsq[:, :, 10:11], na_t)
        # clamp tiny negatives from rounding before sqrt
        nc.vector.tensor_scalar_max(out=dsq[:, :, :], in0=dsq[:, :, :], scalar1=0.0)
        nc.scalar.activation(out=dall[:, :, :], in_=dsq[:, :, :],
                             func=mybir.ActivationFunctionType.Sqrt)
        nc.vector.tensor_reduce(out=dn1m[:, :], in_=dall[:, :, 0:5],
                                axis=mybir.AxisListType.X, op=A.min)
        nc.vector.tensor_reduce(out=dn2m[:, :], in_=dall[:, :, 5:10],
                                axis=mybir.AxisListType.X, op=A.min)
        nc.vector.tensor_add(tsum[:, :], dn1m[:, :], dn2m[:, :])
        nc.vector.tensor_scalar(out=hpre[:, :], in0=dall[:, :, 10], scalar1=2.0,
                                scalar2=MARGIN, op0=A.mult, op1=A.add)
        nc.vector.tensor_sub(lossn[:, :], hpre[:, :], tsum[:, :])
        nc.vector.tensor_scalar_max(out=lossn[:, :], in0=lossn[:, :], scalar1=0.0)

        loss_dst = bass.AP(tensor=loss_ext, offset=0, ap=[[1, 128], [128, NB]])
        nc.sync.dma_start(out=loss_dst, in_=lossn[:, :])

    nc.finalize()
    return nc


def _exact_losses_head(vfeat, afeat, ks):
    """Exact reference loss for anchors in ks (handles the m==k index rewrite)."""
    v = vfeat.astype(np.float64)
    a = afeat.astype(np.float64)
    out = []
    for k in ks:
        idx = [(m + k + 1) % B if m != k else (k + 1) % B for m in range(S)]
        d_p = np.sqrt(np.sum((v[k] - a[k] + EPS) ** 2))
        d1 = min(np.sqrt(np.sum((v[k] - a[j] + EPS) ** 2)) for j in idx)
        d2 = min(np.sqrt(np.sum((a[k] - v[j] + EPS) ** 2)) for j in idx)
        out.append(max(MARGIN + 2.0 * d_p - d1 - d2, 0.0))
    return out


def run_kernel(vfeat, afeat, trace=False):
    from concourse.bass_utils import run_bass_kernel_spmd

    vfeat = np.ascontiguousarray(np.asarray(vfeat, dtype=np.float32))
    afeat = np.ascontiguousarray(np.asarray(afeat, dtype=np.float32))

    if "nc" not in _CACHE:
        _CACHE["nc"] = _build()
    nc = _CACHE["nc"]

    in_maps = []
    for c in range(NCORES):
        lo = c * SH
        idx = np.arange(lo, lo + ROWS) % B
        in_maps.append({"v": vfeat[idx], "a": afeat[idx]})

    res = run_bass_kernel_spmd(nc, in_maps, core_ids=list(range(NCORES)), trace=trace)
    losses = np.concatenate([res.results[c]["loss"] for c in range(NCORES)])

    total = float(np.sum(losses[S:], dtype=np.float64))
    total += sum(_exact_losses_head(vfeat, afeat, range(S)))
    mean = np.float32(total / B)
    return np.asarray(mean, dtype=np.float32), res


def kernel(vfeat, afeat):
    out, _ = run_kernel(vfeat, afeat, trace=False)
    return out
